# revision 18
# baseline (speedup 1.0000x reference)
"""2-layer GCN (GCNConv x2 + log_softmax) on 8 trn2 NeuronCores via Bass/Tile.

Math (identical to the reference by associativity + rank-1 factorization of
the symmetric normalization):
  dis = rsqrt(deg) with self-loops;  A_hat = D^-1/2 (A+I) D^-1/2
  L1: h1 = relu(dis * segsum(T1[src]) + b1),  T1 = (dis*x) @ W1   (16-wide)
  L2: y  = log_softmax((dis * segsum(T2[src])) @ W2 + b2), T2 = dis * h1
Both edge passes are gather+segment-sum of 16-wide rows with identical
structure (W2 is applied AFTER aggregation, so layer 2 also gathers 16-wide).

v5 architecture (feature-major, ap_gather + prefix-scan segment-sum):
  - dis*x folded on host; x shipped bf16; phase A = DMA-transpose + bf16
    matmuls producing q feature-major [16, S] per core.
  - AllGather -> T [128, S]: row 16g+f = feature f of node shard g.
  - The full table lives in SBUF [128, 12500] f32. Edges are partitioned by
    src shard (8 groups); group g's edge list (sorted by dst) is gathered by
    GPSIMD core g via ap_gather (each core reads its own 16 partitions).
  - Segment-sum per group via fp32 prefix scan (tensor_tensor_scan) +
    boundary gather (ap_gather on the scan) + shifted subtract; the 8 group
    partials are combined with one [128x16] selection matmul on PE, which
    also transposes nothing: output is [16, dsts] feature-major.
  - Layer 2's W2 matmul (lhsT = feature-major agg) transposes back to
    node-major for the log_softmax for free.
  - All graph-static data (edge index lists, boundary lists) is baked into
    the NEFF as Const tensors holding all 8 cores' copies; each core pulls
    its slice once per call with a partition-id-offset indirect DMA.
"""

import numpy as np
import ml_dtypes

import concourse.bass as bass
import concourse.mybir as mybir
import concourse.tile as tile
from concourse import library_config
from concourse.vector_clock import ScopedClock

P = 128
F1 = 16
F2 = 40
D = 512
N_NODES = 100000
N_CORES = 8
CH_T = 4           # dst tiles per chunk (n_dst = 512 = one PSUM bank of f32)

# ---------------------------------------------------------------------------
# workaround: this walrus build rejects >1 sync wait per instruction and the
# Drain opcode; spill extra waits onto single-wait nops.
_nop_counter = [0]


def _fresh_nop(engine, wait):
    _nop_counter[0] += 1
    nop = mybir.InstNoOp(name=f"WSPILL-{_nop_counter[0]}", ins=[], outs=[])
    nop.engine = engine
    nop.sync_info = mybir.SyncInfo(on_wait=[wait], on_update=[])
    return nop


def _split_multi_waits(nc):
    for fn in nc.m.functions:
        for bb in fn.blocks:
            insts = bb.instructions
            if not any(
                i.sync_info is not None and len(i.sync_info.on_wait) > 1
                for i in insts
            ):
                continue
            newlist = []
            for inst in insts:
                si = inst.sync_info
                if si is not None and len(si.on_wait) > 1:
                    waits = list(si.on_wait)
                    for w in waits[:-1]:
                        newlist.append(_fresh_nop(inst.engine, w))
                    si.on_wait = waits[-1:]
                    inst.sync_info = si
                newlist.append(inst)
            insts[:] = newlist


def _patched_drain_and_barrier(self, tick_clock, wait_clock):
    nc = self.nc
    drain_inst = nc.sync.nop(nofuse=True, hint="tail_drain_nop")
    wait_clock.add_sem_waits(
        drain_inst.ins, ScopedClock({None: tick_clock.global_clock})
    )
    nc.all_engine_barrier()
    assert self.sems is not None
    popped = nc._tile_sem_poison_stack.pop()
    assert popped is self._sem_poison
    nc.clear_and_free_semaphores(list(self.sems.allocated().values()))
    nc.all_engine_barrier()


tile.TileContext._drain_and_barrier = _patched_drain_and_barrier


def _ceil_to(x, m):
    return (x + m - 1) // m * m


def _wrap16(a):
    """[n] -> wrapped [16, n/16]: logical i at [i%16, i//16]."""
    assert len(a) % 16 == 0
    return np.ascontiguousarray(a.reshape(-1, 16).T)


# ---------------------------------------------------------------------------
def _preprocess_shared(edge_index, N, C):
    """Host-side static layout (shared chunk widths across cores so all
    cores run one SPMD program)."""
    S = N // C
    T = (S + P - 1) // P
    TP = T * P
    n_chunks = (T + CH_T - 1) // CH_T

    e = np.asarray(edge_index)
    src = np.concatenate([e[0], np.arange(N, dtype=e.dtype)]).astype(np.int64)
    dst = np.concatenate([e[1], np.arange(N, dtype=e.dtype)]).astype(np.int64)
    deg = np.bincount(dst, minlength=N).astype(np.int64)
    dis = (1.0 / np.sqrt(np.maximum(deg, 1))).astype(np.float32)
    dis[deg == 0] = 0.0

    # sort edges by (dst, src-shard) once globally
    g = src // S
    order = np.lexsort((g, dst))
    src_s, dst_s, g_s = src[order], dst[order], g[order]

    # counts per (dst, group) -> CSR over the (dst,group) key
    key = dst_s * C + g_s
    cnt = np.bincount(key, minlength=N * C).reshape(N, C)  # [N, C]

    percore = []
    # per-core, per-chunk group slices
    # chunk cc covers local dst cols [cc*CH_T*P, min(...+CH_T*P, TP))
    W = np.zeros(n_chunks, np.int64)   # shared chunk widths
    NB = np.zeros(n_chunks, np.int64)  # boundary list lengths (padded)
    core_data = []
    for c in range(C):
        lo = c * S
        cnt_c = cnt[lo : lo + S]                       # [S, C] counts
        ccum = np.zeros((S + 1, C), np.int64)
        np.cumsum(cnt_c, axis=0, out=ccum[1:])
        # edge rows for this core, grouped by (dst, g): contiguous slice of
        # the global sorted arrays
        lo_e = np.searchsorted(dst_s, lo)
        hi_e = np.searchsorted(dst_s, lo + S)
        srcs_c = src_s[lo_e:hi_e]
        gs_c = g_s[lo_e:hi_e]
        # per group: edges in dst order
        gorder = np.argsort(gs_c, kind="stable")
        srcs_by_g = srcs_c[gorder]
        gcnt = np.bincount(gs_c, minlength=C)
        gptr = np.zeros(C + 1, np.int64)
        np.cumsum(gcnt, out=gptr[1:])
        chunks = []
        for cc in range(n_chunks):
            d0 = cc * CH_T * P
            d1 = min(d0 + CH_T * P, TP)
            dr0, dr1 = min(d0, S), min(d1, S)
            n_dst = d1 - d0
            starts = ccum[dr0:dr1 + 1] - ccum[dr0]     # [n_real+1, C]
            cnts = starts[-1]                          # per-group edge count
            wc = int(cnts.max())
            glists, gbounds = [], []
            for gg in range(C):
                base = gptr[gg] + ccum[dr0, gg]
                lst = srcs_by_g[base : base + cnts[gg]] - gg * S
                glists.append(lst.astype(np.int16))
                b = starts[:, gg]
                if dr1 - dr0 < n_dst:  # pad dst cols (zero-width runs)
                    b = np.concatenate(
                        [b, np.full(n_dst - (dr1 - dr0), b[-1], np.int64)]
                    )
                gbounds.append(b.astype(np.int16))     # [n_dst+1]
            chunks.append((wc, n_dst, glists, gbounds))
            W[cc] = max(W[cc], wc)
        core_data.append(chunks)

    # idx slices must stay 8B-aligned per chunk: pad widths to %64 idxs
    W = np.array([_ceil_to(int(w), 64) for w in W], np.int64)
    NB = np.array(
        [_ceil_to(CH_T * P + 1, 64) for _ in range(n_chunks)], np.int64
    )
    totW16, totB16 = int(W.sum()) // 16, int(NB.sum()) // 16

    for c in range(C):
        midx = np.zeros((P, totW16), np.int16)
        bidx = np.zeros((P, totB16), np.int16)
        wo = bo = 0
        for cc, (wc, n_dst, glists, gbounds) in enumerate(core_data[c]):
            wn, bn = int(W[cc]), int(NB[cc])
            for gg in range(C):
                lst = np.zeros(wn, np.int16)
                lst[: len(glists[gg])] = glists[gg]
                midx[16 * gg : 16 * (gg + 1), wo : wo + wn // 16] = _wrap16(lst)
                b = np.full(bn, gbounds[gg][-1], np.int16)
                b[: n_dst + 1] = gbounds[gg]
                bidx[16 * gg : 16 * (gg + 1), bo : bo + bn // 16] = _wrap16(b)
            wo += wn // 16
            bo += bn // 16

        lo = c * S
        disq = np.ones((F1, TP), np.float32)
        disq[:, :S] = dis[lo : lo + S][None, :]
        percore.append(dict(midx=midx, bidx=bidx, disq=disq))

    meta = dict(
        N=N, C=C, S=S, T=T, n_chunks=n_chunks,
        W=[int(w) for w in W], NB=[int(b) for b in NB],
        totW16=totW16, totB16=totB16,
    )
    return meta, percore, dis


def _build_program(meta, percore):
    N, C, S, T = meta["N"], meta["C"], meta["S"], meta["T"]
    n_chunks, W, NB = meta["n_chunks"], meta["W"], meta["NB"]
    totW16, totB16 = meta["totW16"], meta["totB16"]
    TP = T * P
    fp = mybir.dt.float32
    bf = mybir.dt.bfloat16
    i16 = mybir.dt.int16
    i32 = mybir.dt.int32

    nc = bass.Bass("TRN2", target_bir_lowering=False, debug=False, num_devices=C)
    x_in = nc.declare_dram_parameter("x", [TP, D], bf, isOutput=False)
    w1_in = nc.declare_dram_parameter("W1", [D, F1], bf, isOutput=False)
    b1_in = nc.declare_dram_parameter("b1", [1, F1], fp, isOutput=False)
    w2_in = nc.declare_dram_parameter("W2", [F1, F2], fp, isOutput=False)
    b2_in = nc.declare_dram_parameter("b2", [1, F2], fp, isOutput=False)
    offs_in = nc.declare_dram_parameter("offs", [P, 2], i32, isOutput=False)
    y_out = nc.declare_dram_parameter("y", [TP, F2], bf, isOutput=True)
    import os
    DBG = bool(os.environ.get("DEBUG_V5"))
    if DBG:
        dbg_q = nc.declare_dram_parameter("dbg_q", [F1, S], fp, isOutput=True)
        dbg_tbl = nc.declare_dram_parameter("dbg_tbl", [P, S], fp, isOutput=True)
        dbg_msg = nc.declare_dram_parameter("dbg_msg", [P, 4096], fp, isOutput=True)
        dbg_scn = nc.declare_dram_parameter("dbg_scn", [P, 4096], fp, isOutput=True)
        dbg_G = nc.declare_dram_parameter("dbg_G", [P, 528], fp, isOutput=True)
        dbg_A = nc.declare_dram_parameter("dbg_A", [P, 512], fp, isOutput=True)
        dbg_agg = nc.declare_dram_parameter("dbg_agg", [F1, 512], fp, isOutput=True)
        dbg_midx = nc.declare_dram_parameter("dbg_midx", [P, 256], i16, isOutput=True)
        dbg_u = nc.declare_dram_parameter("dbg_u", [F1, S], fp, isOutput=True)
        dbg_tbl2 = nc.declare_dram_parameter("dbg_tbl2", [P, S], fp, isOutput=True)
        dbg_agg2 = nc.declare_dram_parameter("dbg_agg2", [F1, 512], fp, isOutput=True)

    # graph-static constants: all cores' index lists stacked on axis 0
    c_midx = nc.inline_tensor(
        np.concatenate([pc["midx"] for pc in percore], axis=0), name="c_midx"
    )  # [8*128, totW16]
    c_bidx = nc.inline_tensor(
        np.concatenate([pc["bidx"] for pc in percore], axis=0), name="c_bidx"
    )  # [8*128, totB16]
    c_disq = nc.inline_tensor(
        np.concatenate([pc["disq"] for pc in percore], axis=0), name="c_disq"
    )  # [8*16, TP]
    c_sel = nc.inline_tensor(
        np.ascontiguousarray(np.tile(np.eye(F1, dtype=np.float32), (C, 1))),
        name="c_sel",
    )  # [128, 16]

    q_mine = nc.dram_tensor("q_mine", [F1, S], fp)
    u_mine = nc.dram_tensor("u_mine", [F1, S], fp)
    T1 = nc.dram_tensor("T1", [P, S], fp, addr_space="Shared")
    T2 = nc.dram_tensor("T2", [P, S], fp, addr_space="Shared")
    repl_groups = [list(range(C))]

    Wmax = max(W)
    NBmax = max(NB)

    with tile.TileContext(nc) as tc:
        with tc.tile_pool(name="const", bufs=1) as cpool, \
             tc.tile_pool(name="tb", bufs=1) as tb, \
             tc.tile_pool(name="xtp", bufs=4) as xtp, \
             tc.tile_pool(name="ps", bufs=2, space="PSUM") as ps, \
             tc.tile_pool(name="pt", bufs=2, space="PSUM") as pt, \
             tc.tile_pool(name="pw", bufs=2, space="PSUM") as pw, \
             tc.tile_pool(name="mg", bufs=2) as mg, \
             tc.tile_pool(name="sc", bufs=2) as sc, \
             tc.tile_pool(name="gb", bufs=2) as gb, \
             tc.tile_pool(name="sm", bufs=3) as sm, \
             tc.tile_pool(name="ou", bufs=3) as ou:

            nc.gpsimd.load_library(library_config.ap_gather)

            # --- constants / parameters into SBUF
            offs = cpool.tile([P, 2], i32)
            nc.sync.dma_start(offs[:], offs_in[:, :])
            midx = cpool.tile([P, totW16], i16)
            nc.gpsimd.indirect_dma_start(
                out=midx[:], out_offset=None, in_=c_midx[:, :],
                in_offset=bass.IndirectOffsetOnAxis(ap=offs[:, 0:1], axis=0),
            )
            bidxt = cpool.tile([P, totB16], i16)
            nc.gpsimd.indirect_dma_start(
                out=bidxt[:], out_offset=None, in_=c_bidx[:, :],
                in_offset=bass.IndirectOffsetOnAxis(ap=offs[:, 0:1], axis=0),
            )
            sel = cpool.tile([P, F1], fp)
            nc.sync.dma_start(sel[:], c_sel[:, :])
            w1s = cpool.tile([P, (D // P) * F1], bf)
            nc.sync.dma_start(
                w1s[:].rearrange("p (k f) -> p k f", f=F1),
                w1_in.ap().rearrange("(k p) f -> p k f", p=P),
            )
            w2s = cpool.tile([F1, F2], fp)
            nc.sync.dma_start(w2s[:], w2_in[:, :])
            disq = cpool.tile([F1, TP], fp)
            nc.gpsimd.indirect_dma_start(
                out=disq[:], out_offset=None, in_=c_disq[:, :],
                in_offset=bass.IndirectOffsetOnAxis(ap=offs[0:F1, 1:2], axis=0),
            )
            b1c = cpool.tile([F1, 1], fp)
            nc.sync.dma_start(b1c[:], b1_in.ap().rearrange("o f -> f o"))
            ones_row = cpool.tile([1, P], fp)
            nc.vector.memset(ones_row[:], 1.0)
            b2row = cpool.tile([1, F2], fp)
            nc.sync.dma_start(b2row[:], b2_in[:, :])
            b2ps = pw.tile([P, F2], fp, space="PSUM", tag="wp")
            nc.tensor.matmul(b2ps[:], lhsT=ones_row[:], rhs=b2row[:], start=True, stop=True)
            b2t = cpool.tile([P, F2], fp)
            nc.vector.tensor_copy(b2t[:], b2ps[:])

            # --- phase A: q_fm = (dis*x) @ W1, feature-major [16, S]
            for t in range(T):
                rows = min(P, S - t * P)
                if rows <= 0:
                    break
                psA = pt.tile([F1, P], fp, space="PSUM", tag="pa")
                for k in range(D // P):
                    xts = xtp.tile([P, P], bf, tag="xts")
                    nc.sync.dma_start_transpose(
                        xts[:], x_in[t * P : (t + 1) * P, k * P : (k + 1) * P],
                    )
                    nc.tensor.matmul(
                        psA[:, :], lhsT=w1s[:, k * F1 : (k + 1) * F1],
                        rhs=xts[:, :],
                        start=(k == 0), stop=(k == D // P - 1),
                    )
                qt = sm.tile([F1, P], fp, tag="qt")
                nc.vector.tensor_copy(qt[:], psA[:])
                nc.sync.dma_start(
                    q_mine[:, t * P : t * P + rows], qt[:, :rows]
                )

            nc.gpsimd.collective_compute(
                "AllGather", mybir.AluOpType.bypass, replica_groups=repl_groups,
                ins=[q_mine[:, :]], outs=[T1[:, :]],
            )

            def agg_layer(table, out_cb, dbg=False, dbg2=False):
                tbl = tb.tile([P, S], fp, tag="tbl")
                nc.sync.dma_start(tbl[:], table[:, :])
                if dbg:
                    nc.sync.dma_start(dbg_tbl[:, :], tbl[:])
                if dbg2:
                    nc.sync.dma_start(dbg_tbl2[:, :], tbl[:])
                wo = bo = 0
                for cc in range(n_chunks):
                    wn, bn = W[cc], NB[cc]
                    d0 = cc * CH_T * P
                    n_dst = min(CH_T * P, TP - d0)
                    msg = mg.tile([P, Wmax], fp, tag="msg")
                    nc.gpsimd.ap_gather(
                        msg[:, :wn].rearrange("p (n d) -> p n d", d=1),
                        tbl[:].rearrange("p (n d) -> p n d", d=1),
                        midx[:, wo : wo + wn // 16],
                        P, S, 1, wn,
                    )
                    scn = sc.tile([P, Wmax + 16], fp, tag="scn")
                    nc.vector.memset(scn[:, 0:1], 0.0)
                    nc.vector.tensor_tensor_scan(
                        scn[:, 1 : wn + 1], msg[:, :wn], msg[:, :wn],
                        0.0, op0=mybir.AluOpType.add, op1=mybir.AluOpType.bypass,
                    )
                    G = gb.tile([P, NBmax], fp, tag="G")
                    nc.gpsimd.ap_gather(
                        G[:, :bn].rearrange("p (n d) -> p n d", d=1),
                        scn[:, : wn + 1].rearrange("p (n d) -> p n d", d=1),
                        bidxt[:, bo : bo + bn // 16],
                        P, wn + 1, 1, bn,
                    )
                    A = gb.tile([P, CH_T * P], fp, tag="A")
                    nc.vector.tensor_sub(
                        A[:, :n_dst], G[:, 1 : n_dst + 1], G[:, 0:n_dst]
                    )
                    agg = ps.tile([F1, CH_T * P], fp, space="PSUM", tag="agg")
                    nc.tensor.matmul(
                        agg[:, :n_dst], lhsT=sel[:], rhs=A[:, :n_dst],
                        start=True, stop=True,
                    )
                    if dbg2 and cc == 0:
                        agg2c = sm.tile([F1, CH_T * P], fp, tag="aggc")
                        nc.vector.tensor_copy(agg2c[:, :n_dst], agg[:, :n_dst])
                        nc.sync.dma_start(dbg_agg2[:, :n_dst], agg2c[:, :n_dst])
                    if dbg and cc == 1:
                        nc.sync.dma_start(dbg_msg[:, :min(wn, 4096)], msg[:, :min(wn, 4096)])
                        nc.sync.dma_start(dbg_scn[:, :min(wn + 1, 4096)], scn[:, :min(wn + 1, 4096)])
                        nc.sync.dma_start(dbg_G[:, :bn], G[:, :bn])
                        nc.sync.dma_start(dbg_A[:, :n_dst], A[:, :n_dst])
                        aggc = sm.tile([F1, CH_T * P], fp, tag="aggc")
                        nc.vector.tensor_copy(aggc[:, :n_dst], agg[:, :n_dst])
                        nc.sync.dma_start(dbg_agg[:, :n_dst], aggc[:, :n_dst])
                        nc.sync.dma_start(dbg_midx[:, :min(wn // 16, 256)], midx[:, wo : wo + min(wn // 16, 256)])
                    out_cb(cc, d0, n_dst, agg)
                    wo += wn // 16
                    bo += bn // 16

            def l1_out(cc, d0, n_dst, agg):
                n_real = max(0, min(d0 + n_dst, S) - d0)
                if n_real == 0:
                    return
                t1 = sm.tile([F1, CH_T * P], fp, tag="t1")
                nc.vector.tensor_tensor(
                    t1[:, :n_real], agg[:, :n_real],
                    disq[:, d0 : d0 + n_real], op=mybir.AluOpType.mult,
                )
                nc.vector.tensor_scalar(
                    t1[:, :n_real], t1[:, :n_real], b1c[:, 0:1], 0.0,
                    op0=mybir.AluOpType.add, op1=mybir.AluOpType.max,
                )
                u = sm.tile([F1, CH_T * P], fp, tag="u")
                nc.vector.tensor_tensor(
                    u[:, :n_real], t1[:, :n_real],
                    disq[:, d0 : d0 + n_real], op=mybir.AluOpType.mult,
                )
                nc.sync.dma_start(u_mine[:, d0 : d0 + n_real], u[:, :n_real])

            if DBG:
                nc.sync.dma_start(dbg_q[:, :], q_mine[:, :])
            agg_layer(T1, l1_out, dbg=DBG)

            nc.gpsimd.collective_compute(
                "AllGather", mybir.AluOpType.bypass, replica_groups=repl_groups,
                ins=[u_mine[:, :]], outs=[T2[:, :]],
            )

            def l2_out(cc, d0, n_dst, agg):
                v = sm.tile([F1, CH_T * P], fp, tag="v")
                nc.vector.tensor_tensor(
                    v[:, :n_dst], agg[:, :n_dst],
                    disq[:, d0 : d0 + n_dst], op=mybir.AluOpType.mult,
                )
                for j in range(n_dst // P):
                    r0 = d0 + j * P
                    rows = min(P, S - r0)
                    if rows <= 0:
                        break
                    wp = pw.tile([P, F2], fp, space="PSUM", tag="wp")
                    nc.tensor.matmul(
                        wp[:], lhsT=v[:, j * P : (j + 1) * P], rhs=w2s[:],
                        start=True, stop=True,
                    )
                    w = ou.tile([P, F2], fp, tag="w")
                    nc.vector.tensor_add(w[:], wp[:], b2t[:])
                    mx = sm.tile([P, 1], fp, tag="mx")
                    nc.vector.tensor_reduce(
                        out=mx[:], in_=w[:], op=mybir.AluOpType.max,
                        axis=mybir.AxisListType.X,
                    )
                    nmx = sm.tile([P, 1], fp, tag="nmx")
                    nc.vector.tensor_scalar_mul(nmx[:], mx[:], -1.0)
                    ex = ou.tile([P, F2], fp, tag="ex")
                    se = sm.tile([P, 1], fp, tag="se")
                    nc.scalar.activation(
                        ex[:], w[:], mybir.ActivationFunctionType.Exp,
                        bias=nmx[:], accum_out=se[:],
                    )
                    ls = sm.tile([P, 1], fp, tag="ls")
                    nc.scalar.activation(ls[:], se[:], mybir.ActivationFunctionType.Ln)
                    yt = ou.tile([P, F2], bf, tag="yt")
                    nc.vector.tensor_scalar(
                        yt[:], w[:], mx[:], ls[:],
                        op0=mybir.AluOpType.subtract, op1=mybir.AluOpType.subtract,
                    )
                    nc.sync.dma_start(y_out[r0 : r0 + rows, :], yt[:rows, :])

            if DBG:
                nc.sync.dma_start(dbg_u[:, :], u_mine[:, :])
            agg_layer(T2, l2_out, dbg2=DBG)

    _split_multi_waits(nc)
    mybir.codegen_inst_isa_subclasses(nc)
    return nc


# ---------------------------------------------------------------------------
class _Runner:
    def __init__(self, nc, n_cores):
        import jax
        from jax.sharding import Mesh, PartitionSpec
        from jax.experimental.shard_map import shard_map
        from concourse.bass2jax import (
            _bass_exec_p, partition_id_tensor, install_neuronx_cc_hook,
        )

        install_neuronx_cc_hook()
        self.jax = jax
        self.n_cores = n_cores
        in_names, out_names, out_avals = [], [], []
        partition_name = (
            nc.partition_id_tensor.name if nc.partition_id_tensor else None
        )
        for alloc in nc.m.functions[0].allocations:
            if not isinstance(alloc, mybir.MemoryLocationSet):
                continue
            name = alloc.memorylocations[0].name
            if alloc.kind == "ExternalInput":
                if name != partition_name:
                    in_names.append(name)
            elif alloc.kind == "ExternalOutput":
                out_names.append(name)
                out_avals.append(
                    jax.core.ShapedArray(
                        tuple(alloc.tensor_shape), mybir.dt.np(alloc.dtype)
                    )
                )
        self.in_names, self.out_names, self.out_avals = in_names, out_names, out_avals
        n_params, n_outs = len(in_names), len(out_avals)
        all_in = in_names + out_names
        if partition_name is not None:
            all_in.append(partition_name)

        def _body(*args):
            operands = list(args)
            if partition_name is not None:
                operands.append(partition_id_tensor())
            return tuple(
                _bass_exec_p.bind(
                    *operands, out_avals=tuple(out_avals), in_names=tuple(all_in),
                    out_names=tuple(out_names), lowering_input_output_aliases=(),
                    sim_require_finite=True, sim_require_nnan=True, nc=nc,
                )
            )

        import os
        if os.environ.get("BASS_CPU_SIM"):
            devices = jax.devices("cpu")[:n_cores]
        else:
            devices = jax.devices()[:n_cores]
        mesh = Mesh(np.asarray(devices), ("core",))
        self.fn = jax.jit(
            shard_map(
                _body, mesh=mesh,
                in_specs=(PartitionSpec("core"),) * (n_params + n_outs),
                out_specs=(PartitionSpec("core"),) * n_outs,
                check_rep=False,
            ),
            keep_unused=True,
        )

    def run(self, in_maps):
        concat = [
            np.concatenate([np.asarray(m[name]) for m in in_maps], axis=0)
            for name in self.in_names
        ]
        zeros = [
            np.zeros((self.n_cores * a.shape[0], *a.shape[1:]), a.dtype)
            for a in self.out_avals
        ]
        out = self.fn(*concat, *zeros)
        self.jax.block_until_ready(out)
        res = []
        for c in range(self.n_cores):
            res.append({
                name: np.asarray(out[i]).reshape(
                    self.n_cores, *self.out_avals[i].shape
                )[c]
                for i, name in enumerate(self.out_names)
            })
        return res


_CACHE = {}
_PRE_CACHE = {}


def _make_in_maps(x, W1, b1, W2, b2, percore, C, S, dis):
    T = (S + P - 1) // P
    TP = T * P
    bft = ml_dtypes.bfloat16
    W1b = np.asarray(W1, bft)
    xs_all = (x * dis[:, None]).astype(bft)
    in_maps = []
    for c in range(C):
        pc = percore[c]
        xp_ = np.zeros((TP, D), bft)
        xp_[:S] = xs_all[c * S : (c + 1) * S]
        offs = np.stack([
            c * P + np.arange(P, dtype=np.int32),
            c * F1 + (np.arange(P, dtype=np.int32) % F1),
        ], axis=1).astype(np.int32)
        in_maps.append({
            "x": xp_,
            "W1": W1b, "b1": b1[None], "W2": W2, "b2": b2[None],
            "offs": offs,
        })
    return in_maps


def kernel(x, edge_index, W1, b1, W2, b2):
    x = np.asarray(x, np.float32)
    W1 = np.asarray(W1, np.float32)
    b1 = np.asarray(b1, np.float32)
    W2 = np.asarray(W2, np.float32)
    b2 = np.asarray(b2, np.float32)
    N, C = x.shape[0], N_CORES
    S = N // C

    e_arr = np.asarray(edge_index)
    pkey = (e_arr.shape, int(e_arr[:, :: max(1, e_arr.shape[1] // 1000)].sum()),
            int(e_arr[0, -1]), int(e_arr[1, 0]))
    if pkey not in _PRE_CACHE:
        _PRE_CACHE[pkey] = _preprocess_shared(edge_index, N, C)
    meta, percore, dis = _PRE_CACHE[pkey]
    key = ("gcn5", tuple(meta["W"]))
    if key not in _CACHE:
        nc = _build_program(meta, percore)
        _CACHE[key] = _Runner(nc, C)
    runner = _CACHE[key]

    res = runner.run(_make_in_maps(x, W1, b1, W2, b2, percore, C, S, dis))

    y = np.empty((N, F2), np.float32)
    for c in range(C):
        y[c * S : (c + 1) * S] = res[c]["y"][:S].astype(np.float32)
    return y


# revision 19
# speedup vs baseline: 1.0292x; 1.0292x over previous
"""2-layer GCN (GCNConv x2 + log_softmax) on 8 trn2 NeuronCores via Bass/Tile.

Math (identical to the reference by associativity + rank-1 factorization of
the symmetric normalization):
  dis = rsqrt(deg) with self-loops;  A_hat = D^-1/2 (A+I) D^-1/2
  L1: h1 = relu(dis * segsum(T1[src]) + b1),  T1 = (dis*x) @ W1   (16-wide)
  L2: y  = log_softmax((dis * segsum(T2[src])) @ W2 + b2), T2 = dis * h1
Both edge passes are gather+segment-sum of 16-wide rows with identical
structure (W2 is applied AFTER aggregation, so layer 2 also gathers 16-wide).

v5 architecture (feature-major, ap_gather + prefix-scan segment-sum):
  - dis*x folded on host; x shipped bf16; phase A = DMA-transpose + bf16
    matmuls producing q feature-major [16, S] per core.
  - AllGather -> T [128, S]: row 16g+f = feature f of node shard g.
  - The full table lives in SBUF [128, 12500] f32. Edges are partitioned by
    src shard (8 groups); group g's edge list (sorted by dst) is gathered by
    GPSIMD core g via ap_gather (each core reads its own 16 partitions).
  - Segment-sum per group via fp32 prefix scan (tensor_tensor_scan) +
    boundary gather (ap_gather on the scan) + shifted subtract; the 8 group
    partials are combined with one [128x16] selection matmul on PE, which
    also transposes nothing: output is [16, dsts] feature-major.
  - Layer 2's W2 matmul (lhsT = feature-major agg) transposes back to
    node-major for the log_softmax for free.
  - All graph-static data (edge index lists, boundary lists) is baked into
    the NEFF as Const tensors holding all 8 cores' copies; each core pulls
    its slice once per call with a partition-id-offset indirect DMA.
"""

import numpy as np
import ml_dtypes

import concourse.bass as bass
import concourse.mybir as mybir
import concourse.tile as tile
from concourse import library_config
from concourse.vector_clock import ScopedClock

P = 128
F1 = 16
F2 = 40
D = 512
N_NODES = 100000
N_CORES = 8
CH_T = 4           # dst tiles per chunk (n_dst = 512 = one PSUM bank of f32)

# ---------------------------------------------------------------------------
# workaround: this walrus build rejects >1 sync wait per instruction and the
# Drain opcode; spill extra waits onto single-wait nops.
_nop_counter = [0]


def _fresh_nop(engine, wait):
    _nop_counter[0] += 1
    nop = mybir.InstNoOp(name=f"WSPILL-{_nop_counter[0]}", ins=[], outs=[])
    nop.engine = engine
    nop.sync_info = mybir.SyncInfo(on_wait=[wait], on_update=[])
    return nop


def _split_multi_waits(nc):
    for fn in nc.m.functions:
        for bb in fn.blocks:
            insts = bb.instructions
            if not any(
                i.sync_info is not None and len(i.sync_info.on_wait) > 1
                for i in insts
            ):
                continue
            newlist = []
            for inst in insts:
                si = inst.sync_info
                if si is not None and len(si.on_wait) > 1:
                    waits = list(si.on_wait)
                    for w in waits[:-1]:
                        newlist.append(_fresh_nop(inst.engine, w))
                    si.on_wait = waits[-1:]
                    inst.sync_info = si
                newlist.append(inst)
            insts[:] = newlist


def _patched_drain_and_barrier(self, tick_clock, wait_clock):
    nc = self.nc
    drain_inst = nc.sync.nop(nofuse=True, hint="tail_drain_nop")
    wait_clock.add_sem_waits(
        drain_inst.ins, ScopedClock({None: tick_clock.global_clock})
    )
    nc.all_engine_barrier()
    assert self.sems is not None
    popped = nc._tile_sem_poison_stack.pop()
    assert popped is self._sem_poison
    nc.clear_and_free_semaphores(list(self.sems.allocated().values()))
    nc.all_engine_barrier()


tile.TileContext._drain_and_barrier = _patched_drain_and_barrier


def _ceil_to(x, m):
    return (x + m - 1) // m * m


def _wrap16(a):
    """[n] -> wrapped [16, n/16]: logical i at [i%16, i//16]."""
    assert len(a) % 16 == 0
    return np.ascontiguousarray(a.reshape(-1, 16).T)


# ---------------------------------------------------------------------------
def _preprocess_shared(edge_index, N, C):
    """Host-side static layout (shared chunk widths across cores so all
    cores run one SPMD program)."""
    S = N // C
    T = (S + P - 1) // P
    TP = T * P
    n_chunks = (T + CH_T - 1) // CH_T

    e = np.asarray(edge_index)
    src = np.concatenate([e[0], np.arange(N, dtype=e.dtype)]).astype(np.int64)
    dst = np.concatenate([e[1], np.arange(N, dtype=e.dtype)]).astype(np.int64)
    deg = np.bincount(dst, minlength=N).astype(np.int64)
    dis = (1.0 / np.sqrt(np.maximum(deg, 1))).astype(np.float32)
    dis[deg == 0] = 0.0

    # sort edges by (dst, src-shard) once globally
    g = src // S
    order = np.lexsort((g, dst))
    src_s, dst_s, g_s = src[order], dst[order], g[order]

    # counts per (dst, group) -> CSR over the (dst,group) key
    key = dst_s * C + g_s
    cnt = np.bincount(key, minlength=N * C).reshape(N, C)  # [N, C]

    percore = []
    # per-core, per-chunk group slices
    # chunk cc covers local dst cols [cc*CH_T*P, min(...+CH_T*P, TP))
    W = np.zeros(n_chunks, np.int64)   # shared chunk widths
    NB = np.zeros(n_chunks, np.int64)  # boundary list lengths (padded)
    core_data = []
    for c in range(C):
        lo = c * S
        cnt_c = cnt[lo : lo + S]                       # [S, C] counts
        ccum = np.zeros((S + 1, C), np.int64)
        np.cumsum(cnt_c, axis=0, out=ccum[1:])
        # edge rows for this core, grouped by (dst, g): contiguous slice of
        # the global sorted arrays
        lo_e = np.searchsorted(dst_s, lo)
        hi_e = np.searchsorted(dst_s, lo + S)
        srcs_c = src_s[lo_e:hi_e]
        gs_c = g_s[lo_e:hi_e]
        # per group: edges in dst order
        gorder = np.argsort(gs_c, kind="stable")
        srcs_by_g = srcs_c[gorder]
        gcnt = np.bincount(gs_c, minlength=C)
        gptr = np.zeros(C + 1, np.int64)
        np.cumsum(gcnt, out=gptr[1:])
        chunks = []
        for cc in range(n_chunks):
            d0 = cc * CH_T * P
            d1 = min(d0 + CH_T * P, TP)
            dr0, dr1 = min(d0, S), min(d1, S)
            n_dst = d1 - d0
            starts = ccum[dr0:dr1 + 1] - ccum[dr0]     # [n_real+1, C]
            cnts = starts[-1]                          # per-group edge count
            wc = int(cnts.max())
            glists, gbounds = [], []
            for gg in range(C):
                base = gptr[gg] + ccum[dr0, gg]
                lst = srcs_by_g[base : base + cnts[gg]] - gg * S
                glists.append(lst.astype(np.int16))
                b = starts[:, gg]
                if dr1 - dr0 < n_dst:  # pad dst cols (zero-width runs)
                    b = np.concatenate(
                        [b, np.full(n_dst - (dr1 - dr0), b[-1], np.int64)]
                    )
                gbounds.append(b.astype(np.int16))     # [n_dst+1]
            chunks.append((wc, n_dst, glists, gbounds))
            W[cc] = max(W[cc], wc)
        core_data.append(chunks)

    # idx slices must stay 8B-aligned per chunk: pad widths to %64 idxs
    W = np.array([_ceil_to(int(w), 64) for w in W], np.int64)
    NB = np.array(
        [_ceil_to(CH_T * P + 1, 64) for _ in range(n_chunks)], np.int64
    )
    totW16, totB16 = int(W.sum()) // 16, int(NB.sum()) // 16

    for c in range(C):
        midx = np.zeros((P, totW16), np.int16)
        bidx = np.zeros((P, totB16), np.int16)
        wo = bo = 0
        for cc, (wc, n_dst, glists, gbounds) in enumerate(core_data[c]):
            wn, bn = int(W[cc]), int(NB[cc])
            for gg in range(C):
                lst = np.zeros(wn, np.int16)
                lst[: len(glists[gg])] = glists[gg]
                midx[16 * gg : 16 * (gg + 1), wo : wo + wn // 16] = _wrap16(lst)
                b = np.full(bn, gbounds[gg][-1], np.int16)
                b[: n_dst + 1] = gbounds[gg]
                bidx[16 * gg : 16 * (gg + 1), bo : bo + bn // 16] = _wrap16(b)
            wo += wn // 16
            bo += bn // 16

        lo = c * S
        disq = np.ones((F1, TP), np.float32)
        disq[:, :S] = dis[lo : lo + S][None, :]
        percore.append(dict(midx=midx, bidx=bidx, disq=disq))

    meta = dict(
        N=N, C=C, S=S, T=T, n_chunks=n_chunks,
        W=[int(w) for w in W], NB=[int(b) for b in NB],
        totW16=totW16, totB16=totB16,
    )
    return meta, percore, dis


def _build_program(meta, percore):
    N, C, S, T = meta["N"], meta["C"], meta["S"], meta["T"]
    n_chunks, W, NB = meta["n_chunks"], meta["W"], meta["NB"]
    totW16, totB16 = meta["totW16"], meta["totB16"]
    TP = T * P
    fp = mybir.dt.float32
    bf = mybir.dt.bfloat16
    i16 = mybir.dt.int16
    i32 = mybir.dt.int32

    nc = bass.Bass("TRN2", target_bir_lowering=False, debug=False, num_devices=C)
    x_in = nc.declare_dram_parameter("x", [TP, D], bf, isOutput=False)
    w1_in = nc.declare_dram_parameter("W1", [D, F1], bf, isOutput=False)
    b1_in = nc.declare_dram_parameter("b1", [1, F1], fp, isOutput=False)
    w2_in = nc.declare_dram_parameter("W2", [F1, F2], fp, isOutput=False)
    b2_in = nc.declare_dram_parameter("b2", [1, F2], fp, isOutput=False)
    offs_in = nc.declare_dram_parameter("offs", [P, 2], i32, isOutput=False)
    y_out = nc.declare_dram_parameter("y", [TP, F2], bf, isOutput=True)
    import os
    DBG = bool(os.environ.get("DEBUG_V5"))
    if DBG:
        dbg_q = nc.declare_dram_parameter("dbg_q", [F1, S], fp, isOutput=True)
        dbg_tbl = nc.declare_dram_parameter("dbg_tbl", [P, S], fp, isOutput=True)
        dbg_msg = nc.declare_dram_parameter("dbg_msg", [P, 4096], fp, isOutput=True)
        dbg_scn = nc.declare_dram_parameter("dbg_scn", [P, 4096], fp, isOutput=True)
        dbg_G = nc.declare_dram_parameter("dbg_G", [P, 528], fp, isOutput=True)
        dbg_A = nc.declare_dram_parameter("dbg_A", [P, 512], fp, isOutput=True)
        dbg_agg = nc.declare_dram_parameter("dbg_agg", [F1, 512], fp, isOutput=True)
        dbg_midx = nc.declare_dram_parameter("dbg_midx", [P, 256], i16, isOutput=True)
        dbg_u = nc.declare_dram_parameter("dbg_u", [F1, S], fp, isOutput=True)
        dbg_tbl2 = nc.declare_dram_parameter("dbg_tbl2", [P, S], fp, isOutput=True)
        dbg_agg2 = nc.declare_dram_parameter("dbg_agg2", [F1, 512], fp, isOutput=True)

    # graph-static constants: all cores' index lists stacked on axis 0
    c_midx = nc.inline_tensor(
        np.concatenate([pc["midx"] for pc in percore], axis=0), name="c_midx"
    )  # [8*128, totW16]
    c_bidx = nc.inline_tensor(
        np.concatenate([pc["bidx"] for pc in percore], axis=0), name="c_bidx"
    )  # [8*128, totB16]
    c_disq = nc.inline_tensor(
        np.concatenate([pc["disq"] for pc in percore], axis=0), name="c_disq"
    )  # [8*16, TP]
    c_sel = nc.inline_tensor(
        np.ascontiguousarray(np.tile(np.eye(F1, dtype=np.float32), (C, 1))),
        name="c_sel",
    )  # [128, 16]

    q_mine = nc.dram_tensor("q_mine", [F1, S], fp)
    u_mine = nc.dram_tensor("u_mine", [F1, S], fp)
    T1 = nc.dram_tensor("T1", [P, S], fp, addr_space="Shared")
    T2 = nc.dram_tensor("T2", [P, S], fp, addr_space="Shared")
    repl_groups = [list(range(C))]

    Wmax = max(W)
    NBmax = max(NB)

    with tile.TileContext(nc) as tc:
        with tc.tile_pool(name="const", bufs=1) as cpool, \
             tc.tile_pool(name="tb", bufs=1) as tb, \
             tc.tile_pool(name="xtp", bufs=4) as xtp, \
             tc.tile_pool(name="ps", bufs=2, space="PSUM") as ps, \
             tc.tile_pool(name="pt", bufs=2, space="PSUM") as pt, \
             tc.tile_pool(name="pw", bufs=2, space="PSUM") as pw, \
             tc.tile_pool(name="mg", bufs=2) as mg, \
             tc.tile_pool(name="sc", bufs=2) as sc, \
             tc.tile_pool(name="gb", bufs=2) as gb, \
             tc.tile_pool(name="sm", bufs=3) as sm, \
             tc.tile_pool(name="ou", bufs=3) as ou:

            nc.gpsimd.load_library(library_config.ap_gather)

            # --- constants / parameters into SBUF
            offs = cpool.tile([P, 2], i32)
            nc.sync.dma_start(offs[:], offs_in[:, :])
            midx = cpool.tile([P, totW16], i16)
            nc.gpsimd.indirect_dma_start(
                out=midx[:], out_offset=None, in_=c_midx[:, :],
                in_offset=bass.IndirectOffsetOnAxis(ap=offs[:, 0:1], axis=0),
            )
            bidxt = cpool.tile([P, totB16], i16)
            nc.gpsimd.indirect_dma_start(
                out=bidxt[:], out_offset=None, in_=c_bidx[:, :],
                in_offset=bass.IndirectOffsetOnAxis(ap=offs[:, 0:1], axis=0),
            )
            sel = cpool.tile([P, F1], fp)
            nc.sync.dma_start(sel[:], c_sel[:, :])
            w1s = cpool.tile([P, (D // P) * F1], bf)
            nc.sync.dma_start(
                w1s[:].rearrange("p (k f) -> p k f", f=F1),
                w1_in.ap().rearrange("(k p) f -> p k f", p=P),
            )
            w2s = cpool.tile([F1, F2], fp)
            nc.sync.dma_start(w2s[:], w2_in[:, :])
            disq = cpool.tile([F1, TP], fp)
            nc.gpsimd.indirect_dma_start(
                out=disq[:], out_offset=None, in_=c_disq[:, :],
                in_offset=bass.IndirectOffsetOnAxis(ap=offs[0:F1, 1:2], axis=0),
            )
            b1c = cpool.tile([F1, 1], fp)
            nc.sync.dma_start(b1c[:], b1_in.ap().rearrange("o f -> f o"))
            ones_row = cpool.tile([1, P], fp)
            nc.vector.memset(ones_row[:], 1.0)
            b2row = cpool.tile([1, F2], fp)
            nc.sync.dma_start(b2row[:], b2_in[:, :])
            b2ps = pw.tile([P, F2], fp, space="PSUM", tag="wp")
            nc.tensor.matmul(b2ps[:], lhsT=ones_row[:], rhs=b2row[:], start=True, stop=True)
            b2t = cpool.tile([P, F2], fp)
            nc.vector.tensor_copy(b2t[:], b2ps[:])

            # --- phase A: q_fm = (dis*x) @ W1, feature-major [16, S]
            for t in range(T):
                rows = min(P, S - t * P)
                if rows <= 0:
                    break
                psA = pt.tile([F1, P], fp, space="PSUM", tag="pa")
                for k in range(D // P):
                    xts = xtp.tile([P, P], bf, tag="xts")
                    nc.sync.dma_start_transpose(
                        xts[:], x_in[t * P : (t + 1) * P, k * P : (k + 1) * P],
                    )
                    nc.tensor.matmul(
                        psA[:, :], lhsT=w1s[:, k * F1 : (k + 1) * F1],
                        rhs=xts[:, :],
                        start=(k == 0), stop=(k == D // P - 1),
                    )
                qt = sm.tile([F1, P], fp, tag="qt")
                nc.vector.tensor_copy(qt[:], psA[:])
                nc.sync.dma_start(
                    q_mine[:, t * P : t * P + rows], qt[:, :rows]
                )

            nc.gpsimd.collective_compute(
                "AllGather", mybir.AluOpType.bypass, replica_groups=repl_groups,
                ins=[q_mine[:, :]], outs=[T1[:, :]],
            )

            def agg_layer(table, out_cb, dbg=False, dbg2=False):
                tbl = tb.tile([P, S], fp, tag="tbl")
                nc.sync.dma_start(tbl[:], table[:, :])
                if dbg:
                    nc.sync.dma_start(dbg_tbl[:, :], tbl[:])
                if dbg2:
                    nc.sync.dma_start(dbg_tbl2[:, :], tbl[:])
                wo = bo = 0
                for cc in range(n_chunks):
                    wn, bn = W[cc], NB[cc]
                    d0 = cc * CH_T * P
                    n_dst = min(CH_T * P, TP - d0)
                    msg = mg.tile([P, Wmax], fp, tag="msg")
                    nc.gpsimd.ap_gather(
                        msg[:, :wn].rearrange("p (n d) -> p n d", d=1),
                        tbl[:].rearrange("p (n d) -> p n d", d=1),
                        midx[:, wo : wo + wn // 16],
                        P, S, 1, wn,
                    )
                    scn = sc.tile([P, Wmax + 16], fp, tag="scn")
                    nc.vector.memset(scn[:, 0:1], 0.0)
                    nc.vector.tensor_tensor_scan(
                        scn[:, 1 : wn + 1], msg[:, :wn], msg[:, :wn],
                        0.0, op0=mybir.AluOpType.add, op1=mybir.AluOpType.bypass,
                    )
                    G = gb.tile([P, NBmax], fp, tag="G")
                    nc.gpsimd.ap_gather(
                        G[:, :bn].rearrange("p (n d) -> p n d", d=1),
                        scn[:, : wn + 1].rearrange("p (n d) -> p n d", d=1),
                        bidxt[:, bo : bo + bn // 16],
                        P, wn + 1, 1, bn,
                    )
                    A = gb.tile([P, CH_T * P], fp, tag="A")
                    nc.vector.tensor_sub(
                        A[:, :n_dst], G[:, 1 : n_dst + 1], G[:, 0:n_dst]
                    )
                    agg = ps.tile([F1, CH_T * P], fp, space="PSUM", tag="agg")
                    nc.tensor.matmul(
                        agg[:, :n_dst], lhsT=sel[:], rhs=A[:, :n_dst],
                        start=True, stop=True,
                    )
                    if dbg2 and cc == 0:
                        agg2c = sm.tile([F1, CH_T * P], fp, tag="aggc")
                        nc.vector.tensor_copy(agg2c[:, :n_dst], agg[:, :n_dst])
                        nc.sync.dma_start(dbg_agg2[:, :n_dst], agg2c[:, :n_dst])
                    if dbg and cc == 1:
                        nc.sync.dma_start(dbg_msg[:, :min(wn, 4096)], msg[:, :min(wn, 4096)])
                        nc.sync.dma_start(dbg_scn[:, :min(wn + 1, 4096)], scn[:, :min(wn + 1, 4096)])
                        nc.sync.dma_start(dbg_G[:, :bn], G[:, :bn])
                        nc.sync.dma_start(dbg_A[:, :n_dst], A[:, :n_dst])
                        aggc = sm.tile([F1, CH_T * P], fp, tag="aggc")
                        nc.vector.tensor_copy(aggc[:, :n_dst], agg[:, :n_dst])
                        nc.sync.dma_start(dbg_agg[:, :n_dst], aggc[:, :n_dst])
                        nc.sync.dma_start(dbg_midx[:, :min(wn // 16, 256)], midx[:, wo : wo + min(wn // 16, 256)])
                    out_cb(cc, d0, n_dst, agg)
                    wo += wn // 16
                    bo += bn // 16

            def l1_out(cc, d0, n_dst, agg):
                n_real = max(0, min(d0 + n_dst, S) - d0)
                if n_real == 0:
                    return
                t1 = sm.tile([F1, CH_T * P], fp, tag="t1")
                nc.vector.tensor_tensor(
                    t1[:, :n_real], agg[:, :n_real],
                    disq[:, d0 : d0 + n_real], op=mybir.AluOpType.mult,
                )
                nc.vector.tensor_scalar(
                    t1[:, :n_real], t1[:, :n_real], b1c[:, 0:1], 0.0,
                    op0=mybir.AluOpType.add, op1=mybir.AluOpType.max,
                )
                u = sm.tile([F1, CH_T * P], fp, tag="u")
                nc.vector.tensor_tensor(
                    u[:, :n_real], t1[:, :n_real],
                    disq[:, d0 : d0 + n_real], op=mybir.AluOpType.mult,
                )
                nc.sync.dma_start(u_mine[:, d0 : d0 + n_real], u[:, :n_real])

            if DBG:
                nc.sync.dma_start(dbg_q[:, :], q_mine[:, :])
            agg_layer(T1, l1_out, dbg=DBG)

            nc.gpsimd.collective_compute(
                "AllGather", mybir.AluOpType.bypass, replica_groups=repl_groups,
                ins=[u_mine[:, :]], outs=[T2[:, :]],
            )

            def l2_out(cc, d0, n_dst, agg):
                v = sm.tile([F1, CH_T * P], fp, tag="v")
                nc.vector.tensor_tensor(
                    v[:, :n_dst], agg[:, :n_dst],
                    disq[:, d0 : d0 + n_dst], op=mybir.AluOpType.mult,
                )
                for j in range(n_dst // P):
                    r0 = d0 + j * P
                    rows = min(P, S - r0)
                    if rows <= 0:
                        break
                    wp = pw.tile([P, F2], fp, space="PSUM", tag="wp")
                    nc.tensor.matmul(
                        wp[:], lhsT=v[:, j * P : (j + 1) * P], rhs=w2s[:],
                        start=True, stop=True,
                    )
                    w = ou.tile([P, F2], fp, tag="w")
                    nc.vector.tensor_add(w[:], wp[:], b2t[:])
                    mx = sm.tile([P, 1], fp, tag="mx")
                    nc.vector.tensor_reduce(
                        out=mx[:], in_=w[:], op=mybir.AluOpType.max,
                        axis=mybir.AxisListType.X,
                    )
                    nmx = sm.tile([P, 1], fp, tag="nmx")
                    nc.vector.tensor_scalar_mul(nmx[:], mx[:], -1.0)
                    ex = ou.tile([P, F2], fp, tag="ex")
                    se = sm.tile([P, 1], fp, tag="se")
                    nc.scalar.activation(
                        ex[:], w[:], mybir.ActivationFunctionType.Exp,
                        bias=nmx[:], accum_out=se[:],
                    )
                    ls = sm.tile([P, 1], fp, tag="ls")
                    nc.scalar.activation(ls[:], se[:], mybir.ActivationFunctionType.Ln)
                    yt = ou.tile([P, F2], bf, tag="yt")
                    nc.vector.tensor_scalar(
                        yt[:], w[:], mx[:], ls[:],
                        op0=mybir.AluOpType.subtract, op1=mybir.AluOpType.subtract,
                    )
                    nc.sync.dma_start(y_out[r0 : r0 + rows, :], yt[:rows, :])

            if DBG:
                nc.sync.dma_start(dbg_u[:, :], u_mine[:, :])
            agg_layer(T2, l2_out, dbg2=DBG)

    _split_multi_waits(nc)
    mybir.codegen_inst_isa_subclasses(nc)
    return nc


# ---------------------------------------------------------------------------
class _Runner:
    def __init__(self, nc, n_cores):
        import jax
        from jax.sharding import Mesh, PartitionSpec
        from jax.experimental.shard_map import shard_map
        from concourse.bass2jax import (
            _bass_exec_p, partition_id_tensor, install_neuronx_cc_hook,
        )

        install_neuronx_cc_hook()
        self.jax = jax
        self.n_cores = n_cores
        in_names, out_names, out_avals = [], [], []
        partition_name = (
            nc.partition_id_tensor.name if nc.partition_id_tensor else None
        )
        for alloc in nc.m.functions[0].allocations:
            if not isinstance(alloc, mybir.MemoryLocationSet):
                continue
            name = alloc.memorylocations[0].name
            if alloc.kind == "ExternalInput":
                if name != partition_name:
                    in_names.append(name)
            elif alloc.kind == "ExternalOutput":
                out_names.append(name)
                out_avals.append(
                    jax.core.ShapedArray(
                        tuple(alloc.tensor_shape), mybir.dt.np(alloc.dtype)
                    )
                )
        self.in_names, self.out_names, self.out_avals = in_names, out_names, out_avals
        n_params, n_outs = len(in_names), len(out_avals)
        all_in = in_names + out_names
        if partition_name is not None:
            all_in.append(partition_name)

        def _body(*args):
            operands = list(args)
            if partition_name is not None:
                operands.append(partition_id_tensor())
            return tuple(
                _bass_exec_p.bind(
                    *operands, out_avals=tuple(out_avals), in_names=tuple(all_in),
                    out_names=tuple(out_names), lowering_input_output_aliases=(),
                    sim_require_finite=True, sim_require_nnan=True, nc=nc,
                )
            )

        import os
        if os.environ.get("BASS_CPU_SIM"):
            devices = jax.devices("cpu")[:n_cores]
        else:
            devices = jax.devices()[:n_cores]
        mesh = Mesh(np.asarray(devices), ("core",))
        self.fn = jax.jit(
            shard_map(
                _body, mesh=mesh,
                in_specs=(PartitionSpec("core"),) * (n_params + n_outs),
                out_specs=(PartitionSpec("core"),) * n_outs,
                check_rep=False,
            ),
            keep_unused=True,
        )

    def run(self, in_maps):
        concat = [
            np.concatenate([np.asarray(m[name]) for m in in_maps], axis=0)
            for name in self.in_names
        ]
        zeros = [
            np.zeros((self.n_cores * a.shape[0], *a.shape[1:]), a.dtype)
            for a in self.out_avals
        ]
        out = self.fn(*concat, *zeros)
        self.jax.block_until_ready(out)
        res = []
        for c in range(self.n_cores):
            res.append({
                name: np.asarray(out[i]).reshape(
                    self.n_cores, *self.out_avals[i].shape
                )[c]
                for i, name in enumerate(self.out_names)
            })
        return res


_CACHE = {}
_PRE_CACHE = {}


def _make_in_maps(x, W1, b1, W2, b2, percore, C, S, dis):
    T = (S + P - 1) // P
    TP = T * P
    bft = ml_dtypes.bfloat16
    W1b = np.asarray(W1, bft)
    xs_all = (x * dis[:, None]).astype(bft)
    in_maps = []
    for c in range(C):
        pc = percore[c]
        xp_ = np.zeros((TP, D), bft)
        xp_[:S] = xs_all[c * S : (c + 1) * S]
        offs = np.stack([
            c * P + np.arange(P, dtype=np.int32),
            c * F1 + (np.arange(P, dtype=np.int32) % F1),
        ], axis=1).astype(np.int32)
        in_maps.append({
            "x": xp_,
            "W1": W1b, "b1": b1[None], "W2": W2, "b2": b2[None],
            "offs": offs,
        })
    return in_maps


def kernel(x, edge_index, W1, b1, W2, b2):
    x = np.asarray(x, np.float32)
    W1 = np.asarray(W1, np.float32)
    b1 = np.asarray(b1, np.float32)
    W2 = np.asarray(W2, np.float32)
    b2 = np.asarray(b2, np.float32)
    N, C = x.shape[0], N_CORES
    S = N // C

    e_arr = np.asarray(edge_index)
    pkey = (e_arr.shape, int(e_arr[:, :: max(1, e_arr.shape[1] // 1000)].sum()),
            int(e_arr[0, -1]), int(e_arr[1, 0]))
    if pkey not in _PRE_CACHE:
        _PRE_CACHE[pkey] = _preprocess_shared(edge_index, N, C)
    meta, percore, dis = _PRE_CACHE[pkey]
    key = ("gcn5", pkey, tuple(meta["W"]))
    if key not in _CACHE:
        nc = _build_program(meta, percore)
        _CACHE[key] = _Runner(nc, C)
    runner = _CACHE[key]

    res = runner.run(_make_in_maps(x, W1, b1, W2, b2, percore, C, S, dis))

    y = np.empty((N, F2), np.float32)
    for c in range(C):
        y[c * S : (c + 1) * S] = res[c]["y"][:S].astype(np.float32)
    return y
